# revision 9
# baseline (speedup 1.0000x reference)
"""Gated attention layer (B=8, S=2048, D=1024) on 8 Trainium2 NeuronCores.

Sharding: data-parallel over batch B — core b computes batch element b
end-to-end (weights replicated). No collectives.

Per-core dataflow:
  phase 1 (slab-structured, order K -> Q+gate -> V):
    K section: per 256-col slab of Xk^T: PE-transpose 2 x-tiles into the
      slab buffer (fp32), then K^T slab = Wk^T x slab (fp32r), evicted
      into the SBUF-resident K^T.  Transposes for slab i+1 are emitted
      before the projection of slab i so PE never starves.  Wk and Wq
      prefetch on the ACT HWDGE queue while Xk streams on the SP queue.
    Q+gate section: slabs of Xq^T feed both Q^T (fp32 -> DRAM scratch)
      and gate^T = sigmoid(Wg^T Xq^T) (bf16 -> DRAM scratch).  Wg loads
      at section start; Wv's fp32 bits stage into the still-dead V
      buffer so the V section can start without a weight-load bubble.
    V section: Xv tiles are cast to bf16 and DMA-xbar-transposed into
      Xv^T (no PE work), then V = Xv Wv in natural layout (bf16).
      Wo loads/casts here and phase 2's first Q^T tile prefetches under
      this section's matmuls.
  phase 2: blocks of 4 q tiles.  Per q tile: scores = Q^T slice x K^T
      (fp32r, PSUM), softmax along the free axis (DVE per-bank
      max-reduce + ACT exp with fused bias/row-sum, then a cheap
      exp(m_nb - M) cross-bank correction so score banks free early),
      then ONE DMA-xbar transpose of the bf16 attention tile into the
      block buffer (no PE transposes).  Per block: ctx^T = V^T x attnT
      (bf16) with the gate^T multiply fused into eviction, then
      out = ctxgT x Wo (bf16) with the 1/sum normalization fused into
      the final eviction.  Head of block b+1 is emitted before the
      tail of block b so PE never waits on the softmax chain.
"""

import numpy as np

import concourse.bass as bass
import concourse.tile as tile
from concourse import bacc, mybir
from concourse.bass_utils import run_bass_kernel_spmd
from concourse.masks import make_identity

B, S, D = 8, 2048, 1024
P = 128
DK = D // P      # 8 contraction chunks of 128
ST = S // P      # 16 seq tiles of 128
NB = S // 512    # 4 scores banks of 512
SLW = 256        # slab width (columns of X^T per slab)
NSL = S // SLW   # 8 slabs per input
TPS = SLW // P   # 2 s-tiles per slab

F32 = mybir.dt.float32
F32R = mybir.dt.float32r
BF16 = mybir.dt.bfloat16
AX = mybir.AxisListType
ALU = mybir.AluOpType
ACTF = mybir.ActivationFunctionType


def _mm(nc, out, lhsT, rhs, start, stop):
    nc.tensor.matmul(out, lhsT, rhs, start=start, stop=stop)


def build_program(zero_bias: bool, debug: bool = False, reps: int = 1, phases=(1, 2)):
    assert zero_bias, "device path supports zero biases only"
    nc = bacc.Bacc(None, target_bir_lowering=False, debug=debug)

    xq = nc.dram_tensor("xq", [S, D], F32, kind="ExternalInput")
    xk = nc.dram_tensor("xk", [S, D], F32, kind="ExternalInput")
    xv = nc.dram_tensor("xv", [S, D], F32, kind="ExternalInput")
    ws = {
        n: nc.dram_tensor(n, [D, D], F32, kind="ExternalInput")
        for n in ("wq", "wk", "wv", "wg", "wo")
    }
    out = nc.dram_tensor("out", [S, D], F32, kind="ExternalOutput")

    with tile.TileContext(nc) as tc:
        if reps == 1:
            _body(tc, xq, xk, xv, ws, out, phases)
        else:
            with tc.For_i(0, reps, 1):
                _body(tc, xq, xk, xv, ws, out, phases)
    nc.compile()
    return nc


def _body(tc, xq, xk, xv, ws, out, phases=(1, 2)):
    nc = tc.nc
    from contextlib import ExitStack

    with ExitStack() as ctx:
        ep = ctx.enter_context

        dram = ep(tc.tile_pool(name="dram", bufs=1, space="DRAM"))
        qt_dram = dram.tile([P, DK, S], F32)       # Q^T scratch: [d, s]
        gt_dram = dram.tile([P, DK, S], BF16)      # gate^T scratch: [d, s]

        const = ep(tc.tile_pool(name="const", bufs=1))
        ident_f = const.tile([P, P], F32)
        make_identity(nc, ident_f)

        # ---- long-lived SBUF residents (96 KiB/part) ----
        kv_pool = ep(tc.tile_pool(name="kv", bufs=1))
        kT_sb = kv_pool.tile([P, DK, S], F32R)     # K^T  (64 KiB/part)
        v_sb = kv_pool.tile([P, ST, D], BF16)      # V natural (32 KiB/part)

        # =================== phase 1 ===================
        if 1 not in phases:
            nc.vector.memset(kT_sb, 0.0)
            nc.vector.memset(v_sb, 0.0)
        if 1 in phases:
          with tc.tile_pool(name="xslab", bufs=2) as slab_pool, \
               tc.tile_pool(name="xstage", bufs=4) as x_pool, \
               tc.tile_pool(name="wchunk", bufs=16) as w_pool, \
               tc.tile_pool(name="evict", bufs=3, space="PSUM") as evict_pool, \
               tc.tile_pool(name="tp", bufs=4, space="PSUM") as tp_pool:

              def load_w(name):
                  # weights ride the ACT HWDGE queue so Xk/Xq tile loads
                  # on the SP queue are never stuck behind a 4MB transfer
                  tiles = []
                  for k in range(DK):
                      wt = w_pool.tile([P, D], F32R, tag="wchunk")
                      nc.scalar.dma_start(
                          out=wt, in_=ws[name][k * P:(k + 1) * P, :].bitcast(F32R))
                      tiles.append(wt)
                  return tiles

              def make_slab(x_dram, sl):
                  """load TPS x-tiles of slab sl, PE-transpose into a slab
                  buffer [P, DK, SLW] (fp32r)"""
                  slab = slab_pool.tile([P, DK, SLW], F32R, tag="slab")
                  for st in range(TPS):
                      s = sl * TPS + st
                      xt = x_pool.tile([P, D], F32, tag="xstage")
                      nc.sync.dma_start(out=xt, in_=x_dram[s * P:(s + 1) * P, :])
                      for j in range(2):
                          pst = tp_pool.tile([P, 512], F32, tag="tp")
                          for i in range(4):
                              k = j * 4 + i
                              nc.tensor.transpose(
                                  pst[:, i * P:(i + 1) * P],
                                  xt[:, k * P:(k + 1) * P], ident_f)
                          dst = slab[:, j * 4:(j + 1) * 4, st * P:(st + 1) * P]
                          pr = pst.rearrange("p (a b) -> p a b", a=4)
                          nc.vector.tensor_copy(dst, pr)
                  return slab

              # -------- K section: K^T projection (fp32 resident) ---------
              wk = load_w("wk")
              wq = load_w("wq")      # prefetch under the K section
              slab = make_slab(xk, 0)
              for sl in range(NSL):
                  nslab = make_slab(xk, sl + 1) if sl + 1 < NSL \
                      else make_slab(xq, 0)
                  for m in range(DK):
                      ps = evict_pool.tile([P, 512], F32, tag="proj")
                      for k in range(DK):
                          _mm(nc, ps[:, :SLW], wk[k][:, m * P:(m + 1) * P],
                              slab[:, k, :], start=(k == 0), stop=(k == DK - 1))
                      nc.vector.tensor_copy(
                          kT_sb[:, m, sl * SLW:(sl + 1) * SLW], ps[:, :SLW])
                  slab = nslab

              # -------- Q+gate section (shared slabs) ---------------------
              wg = load_w("wg")
              # stage Wv's fp32 bits into the still-dead V buffer (ACT queue)
              wv_staged = []
              for k in range(DK):
                  dstv = v_sb[:, 2 * k:2 * k + 2, :].bitcast(F32)
                  nc.scalar.dma_start(
                      out=dstv,
                      in_=ws["wv"][k * P:(k + 1) * P, :].rearrange(
                          "p (a b) -> p a b", a=2))
                  wv_staged.append(dstv)

              with tc.tile_pool(name="qstage", bufs=3) as q_pool, \
                   tc.tile_pool(name="gstage", bufs=3) as g_pool:
                  for sl in range(NSL):
                      nslab = make_slab(xq, sl + 1) if sl + 1 < NSL else None
                      for m in range(DK):
                          ps = evict_pool.tile([P, 512], F32, tag="proj")
                          for k in range(DK):
                              _mm(nc, ps[:, :SLW], wq[k][:, m * P:(m + 1) * P],
                                  slab[:, k, :], start=(k == 0),
                                  stop=(k == DK - 1))
                          stg = q_pool.tile([P, SLW], F32, tag="qstage")
                          nc.vector.tensor_copy(stg, ps[:, :SLW])
                          # stores ride the ACT queue: a store waiting on its
                          # stage copy must not block x-tile loads on SP
                          nc.scalar.dma_start(
                              out=qt_dram[:, m, sl * SLW:(sl + 1) * SLW],
                              in_=stg)
                      for m in range(DK):
                          ps = evict_pool.tile([P, 512], F32, tag="proj")
                          for k in range(DK):
                              _mm(nc, ps[:, :SLW], wg[k][:, m * P:(m + 1) * P],
                                  slab[:, k, :], start=(k == 0),
                                  stop=(k == DK - 1))
                          stg = g_pool.tile([P, SLW], BF16, tag="gstage")
                          nc.scalar.activation(stg, ps[:, :SLW], ACTF.Sigmoid)
                          nc.scalar.dma_start(
                              out=gt_dram[:, m, sl * SLW:(sl + 1) * SLW],
                              in_=stg)
                      slab = nslab

        # Wo bf16 resident: pool created after the K/QG pools close
        wo_pool = ep(tc.tile_pool(name="wo", bufs=1))
        wo_sb = wo_pool.tile([P, DK, D], BF16)     # 16 KiB/part
        if 1 not in phases:
            nc.vector.memset(wo_sb, 0.0)

        if 1 in phases:
          # -------- V section: bf16, DMA-xbar transposed input ----------
          with tc.tile_pool(name="wvbf", bufs=1) as wvbf_pool, \
               tc.tile_pool(name="wostage", bufs=2) as wo_stage, \
               tc.tile_pool(name="xvT", bufs=1) as xvT_pool, \
               tc.tile_pool(name="xvstage", bufs=3) as xv_pool, \
               tc.tile_pool(name="xvbf", bufs=3) as xvbf_pool, \
               tc.tile_pool(name="evict2", bufs=4, space="PSUM") as ev2_pool:
              # cast the staged Wv bits (in v_sb) to bf16
              wv_bf = wvbf_pool.tile([P, DK, D], BF16)
              for k in range(DK):
                  nc.vector.tensor_copy(
                      wv_bf[:, k, :].rearrange("p (a b) -> p a b", a=2),
                      wv_staged[k])

              xvT = xvT_pool.tile([P, DK, S], BF16)
              for s in range(ST):
                  xt = xv_pool.tile([P, D], F32, tag="xvstage")
                  nc.sync.dma_start(out=xt, in_=xv[s * P:(s + 1) * P, :])
                  xb = xvbf_pool.tile([P, D], BF16, tag="xvbf")
                  nc.vector.tensor_copy(xb, xt)
                  # transpose waits on the cast; keep it off the SP queue so
                  # it cannot head-of-line block the xv tile loads
                  nc.scalar.dma_start(
                      out=xvT[:, :, s * P:(s + 1) * P], in_=xb, transpose=True)

              # Wo load + bf16 cast (ACT queue DMA, DVE cast)
              for k in range(DK):
                  wt = wo_stage.tile([P, D], F32, tag="wostage")
                  nc.scalar.dma_start(out=wt, in_=ws["wo"][k * P:(k + 1) * P, :])
                  nc.vector.tensor_copy(wo_sb[:, k, :], wt)

              for s in range(ST):
                  pss = [ev2_pool.tile([P, 512], F32, tag="vproj",
                                       name=f"pss{_n}") for _n in range(2)]
                  for k in range(DK):
                      for n in range(2):
                          _mm(nc, pss[n], xvT[:, k, s * P:(s + 1) * P],
                              wv_bf[:, k, n * 512:(n + 1) * 512],
                              start=(k == 0), stop=(k == DK - 1))
                  for n in range(2):
                      nc.vector.tensor_copy(
                          v_sb[:, s, n * 512:(n + 1) * 512], pss[n])

        # =================== phase 2 ===================
        if 2 not in phases:
            return

        QB = 4                      # q tiles per block
        NBLK = ST // QB
        attnp = ep(tc.tile_pool(name="attnp", bufs=2))
        outp = ep(tc.tile_pool(name="outp", bufs=3))
        blkp = ep(tc.tile_pool(name="blkp", bufs=1))
        gtp = ep(tc.tile_pool(name="gtp", bufs=2))
        qtp = ep(tc.tile_pool(name="qtp", bufs=2))
        stats = ep(tc.tile_pool(name="stats", bufs=2 * QB + 2))
        ps_a = ep(tc.tile_pool(name="ps_a", bufs=5, space="PSUM"))
        ps_b = ep(tc.tile_pool(name="ps_b", bufs=3, space="PSUM"))

        qt_tiles = {}

        def load_qt(t):
            qt_sb = qtp.tile([P, DK, P], F32R, tag="qt", name="qt_sb")
            nc.scalar.dma_start(
                out=qt_sb, in_=qt_dram[:, :, t * P:(t + 1) * P].bitcast(F32R))
            qt_tiles[t] = qt_sb

        def head(t, attnT_blk, tq):
            """scores + per-bank softmax + DMA-transpose into attnT col tq"""
            # prefetch the NEXT tile's Q slice first: the dma issue must
            # precede this tile's exp instructions on the ACT queue, which
            # wait on scores and would head-of-line block the load
            if t + 1 < ST:
                load_qt(t + 1)
            qt_sb = qt_tiles.pop(t)

            negmax4 = stats.tile([P, NB], F32, tag="negmax4", name="negmax4")
            sums4 = stats.tile([P, NB], F32, tag="sums4", name="sums4")
            neg_max = stats.tile([P, 1], F32, tag="negmax", name="neg_max")
            c4 = stats.tile([P, NB], F32, tag="c4", name="c4")
            recip = stats.tile([P, 1], F32, tag="recip", name="recip")
            sumx = stats.tile([P, 1], F32, tag="sumx", name="sumx")

            score_ps = [ps_a.tile([P, 512], F32, tag="ps_a", name=f"sps{_n}")
                        for _n in range(NB)]
            for k in range(DK):
                for nb in range(NB):
                    _mm(nc, score_ps[nb], qt_sb[:, k, :],
                        kT_sb[:, k, nb * 512:(nb + 1) * 512],
                        start=(k == 0), stop=(k == DK - 1))
            attn = attnp.tile([P, S], BF16, tag="attn", name="attn")
            for nb in range(NB):
                nc.vector.tensor_reduce(
                    negmax4[:, nb:nb + 1], score_ps[nb], axis=AX.X,
                    op=ALU.max, negate=True)
                # exp with per-bank max: frees the psum bank without waiting
                # for the global row max
                nc.scalar.activation(
                    attn[:, nb * 512:(nb + 1) * 512], score_ps[nb], ACTF.Exp,
                    bias=negmax4[:, nb:nb + 1], accum_out=sums4[:, nb:nb + 1])
            # global max + per-bank correction c4 = exp(m_nb - M)
            nc.vector.tensor_reduce(neg_max, negmax4, axis=AX.X, op=ALU.min)
            nc.vector.tensor_scalar(
                out=c4, in0=negmax4, scalar1=neg_max, scalar2=None,
                op0=ALU.subtract)
            nc.scalar.activation(c4, c4, ACTF.Exp, scale=-1.0)
            nc.vector.tensor_tensor(out=sums4, in0=sums4, in1=c4, op=ALU.mult)
            nc.vector.tensor_reduce(sumx, sums4, axis=AX.X, op=ALU.add)
            nc.vector.reciprocal(recip, sumx)
            for nb in range(NB):
                nc.vector.tensor_scalar_mul(
                    attn[:, nb * 512:(nb + 1) * 512],
                    attn[:, nb * 512:(nb + 1) * 512], c4[:, nb:nb + 1])

            # one DMA xbar transpose: attnT_blk[p, kb, tq*P+j] = attn[j, kb*P+p]
            nc.sync.dma_start(
                out=attnT_blk[:, :, tq * P:(tq + 1) * P], in_=attn,
                transpose=True)
            return recip

        def tail_block(blk, attnT_blk, recips):
            q0 = blk * QB * P           # block q offset
            gt_sb = gtp.tile([P, DK, QB * P], BF16, tag="gt", name="gt_sb")
            nc.scalar.dma_start(out=gt_sb, in_=gt_dram[:, :, q0:q0 + QB * P])

            # ctx^T = V^T x attnT (bf16), evict fused with gate^T multiply
            ctxgT_blk = blkp.tile([P, DK, QB * P], BF16, name="ctxgT_blk")
            for mp in range(DK // 2):
                ps_c = [ps_b.tile([P, 512], F32, tag="ps_b", name=f"psc{_n}")
                        for _n in range(2)]
                for kb in range(ST):
                    for h in range(2):
                        m = mp * 2 + h
                        _mm(nc, ps_c[h], v_sb[:, kb, m * P:(m + 1) * P],
                            attnT_blk[:, kb, :],
                            start=(kb == 0), stop=(kb == ST - 1))
                for h in range(2):
                    m = mp * 2 + h
                    nc.vector.tensor_tensor(
                        out=ctxgT_blk[:, m, :], in0=ps_c[h],
                        in1=gt_sb[:, m, :], op=ALU.mult)

            # out = (ctxgT x Wo) * recip, per q tile
            for tq in range(QB):
                t = blk * QB + tq
                ps_o = [ps_b.tile([P, 512], F32, tag="ps_b", name=f"pso{_n}")
                        for _n in range(2)]
                for k in range(DK):
                    for n in range(2):
                        _mm(nc, ps_o[n],
                            ctxgT_blk[:, k, tq * P:(tq + 1) * P],
                            wo_sb[:, k, n * 512:(n + 1) * 512],
                            start=(k == 0), stop=(k == DK - 1))
                for n in range(2):
                    out_sb = outp.tile([P, 512], F32, tag="out", name="out_sb")
                    nc.vector.tensor_scalar_mul(out_sb, ps_o[n], recips[tq])
                    nc.sync.dma_start(
                        out=out[t * P:(t + 1) * P, n * 512:(n + 1) * 512],
                        in_=out_sb)

        prev = None
        load_qt(0)
        for blk in range(NBLK):
            attnT_blk = blkp.tile([P, ST, QB * P], BF16, name="attnT_blk",
                                  tag=f"attnT{blk % 2}")
            recips = []
            for tq in range(QB):
                recips.append(head(blk * QB + tq, attnT_blk, tq))
            if prev is not None:
                tail_block(*prev)
            prev = (blk, attnT_blk, recips)
        tail_block(*prev)


_CACHE = {}


def _get_program(zero_bias: bool):
    if zero_bias not in _CACHE:
        _CACHE[zero_bias] = build_program(zero_bias)
    return _CACHE[zero_bias]


def kernel(queries, keys, values, Wq, bq, Wk, bk, Wv, bv, Wg, bg, Wo, bo):
    queries = np.ascontiguousarray(np.asarray(queries, dtype=np.float32))
    keys = np.ascontiguousarray(np.asarray(keys, dtype=np.float32))
    values = np.ascontiguousarray(np.asarray(values, dtype=np.float32))
    wdict = {
        "wq": np.ascontiguousarray(np.asarray(Wq, np.float32)),
        "wk": np.ascontiguousarray(np.asarray(Wk, np.float32)),
        "wv": np.ascontiguousarray(np.asarray(Wv, np.float32)),
        "wg": np.ascontiguousarray(np.asarray(Wg, np.float32)),
        "wo": np.ascontiguousarray(np.asarray(Wo, np.float32)),
    }
    bdict = {
        "bq": np.ascontiguousarray(np.asarray(bq, np.float32)),
        "bk": np.ascontiguousarray(np.asarray(bk, np.float32)),
        "bv": np.ascontiguousarray(np.asarray(bv, np.float32)),
        "bg": np.ascontiguousarray(np.asarray(bg, np.float32)),
        "bo": np.ascontiguousarray(np.asarray(bo, np.float32)),
    }
    zero_bias = all(not np.any(v) for v in bdict.values())
    if not zero_bias:
        # Bias-enabled device path is not wired up; the problem's
        # setup_inputs() uses all-zero biases, so this branch only exists
        # for off-spec inputs. Compute on host for correctness.
        return _host_reference(queries, keys, values, wdict, bdict)
    nc = _get_program(True)

    in_maps = []
    for b in range(B):
        m = {"xq": queries[b], "xk": keys[b], "xv": values[b]}
        m.update(wdict)
        in_maps.append(m)
    res = run_bass_kernel_spmd(nc, in_maps, core_ids=list(range(B)))
    return np.stack([res.results[b]["out"] for b in range(B)], axis=0)


def _host_reference(queries, keys, values, w, bdict):
    out = np.empty_like(queries)
    for b in range(B):
        q = queries[b] @ w["wq"] + bdict["bq"]
        k = keys[b] @ w["wk"] + bdict["bk"]
        v = values[b] @ w["wv"] + bdict["bv"]
        s = q @ k.T
        s -= s.max(axis=-1, keepdims=True)
        e = np.exp(s)
        a = e / e.sum(axis=-1, keepdims=True)
        gate = 1.0 / (1.0 + np.exp(-(queries[b] @ w["wg"] + bdict["bg"])))
        out[b] = ((a @ v) * gate) @ w["wo"] + bdict["bo"]
    return out


# revision 15
# speedup vs baseline: 1.0253x; 1.0253x over previous
"""Gated attention layer (B=8, S=2048, D=1024) on 8 Trainium2 NeuronCores.

Sharding: data-parallel over batch B — core b computes batch element b
end-to-end (weights replicated). No collectives.

Per-core dataflow:
  phase 1 (slab-structured, order K -> Q+gate -> V):
    K section: per 256-col slab of Xk^T: PE-transpose 2 x-tiles into the
      slab buffer (fp32), then K^T slab = Wk^T x slab (fp32r), evicted
      into the SBUF-resident K^T.  Transposes for slab i+1 are emitted
      before the projection of slab i so PE never starves.  Wk and Wq
      prefetch on the ACT HWDGE queue while Xk streams on the SP queue.
    Q+gate section: slabs of Xq^T feed both Q^T (fp32 -> DRAM scratch)
      and gate^T = sigmoid(Wg^T Xq^T) (bf16 -> DRAM scratch).  Wg loads
      at section start; Wv's fp32 bits stage into the still-dead V
      buffer so the V section can start without a weight-load bubble.
    V section: Xv tiles are cast to bf16 and DMA-xbar-transposed into
      Xv^T (no PE work), then V = Xv Wv in natural layout (bf16).
      Wo loads/casts here and phase 2's first Q^T tile prefetches under
      this section's matmuls.
  phase 2: blocks of 4 q tiles.  Per q tile: scores = Q^T slice x K^T
      (fp32r, PSUM), softmax along the free axis (DVE per-bank
      max-reduce + ACT exp with fused bias/row-sum, then a cheap
      exp(m_nb - M) cross-bank correction so score banks free early),
      then ONE DMA-xbar transpose of the bf16 attention tile into the
      block buffer (no PE transposes).  Per block: ctx^T = V^T x attnT
      (bf16) with the gate^T multiply fused into eviction, then
      out = ctxgT x Wo (bf16) with the 1/sum normalization fused into
      the final eviction.  Head of block b+1 is emitted before the
      tail of block b so PE never waits on the softmax chain.
"""

import numpy as np

import concourse.bass as bass
import concourse.tile as tile
from concourse import bacc, mybir
from concourse.bass_utils import run_bass_kernel_spmd
from concourse.masks import make_identity

B, S, D = 8, 2048, 1024
P = 128
DK = D // P      # 8 contraction chunks of 128
ST = S // P      # 16 seq tiles of 128
NB = S // 512    # 4 scores banks of 512
SLW = 256        # slab width (columns of X^T per slab)
NSL = S // SLW   # 8 slabs per input
TPS = SLW // P   # 2 s-tiles per slab

F32 = mybir.dt.float32
F32R = mybir.dt.float32r
BF16 = mybir.dt.bfloat16
AX = mybir.AxisListType
ALU = mybir.AluOpType
ACTF = mybir.ActivationFunctionType


def _mm(nc, out, lhsT, rhs, start, stop):
    nc.tensor.matmul(out, lhsT, rhs, start=start, stop=stop)


def build_program(zero_bias: bool, debug: bool = False, reps: int = 1, phases=(1, 2)):
    assert zero_bias, "device path supports zero biases only"
    nc = bacc.Bacc(None, target_bir_lowering=False, debug=debug)

    xq = nc.dram_tensor("xq", [S, D], F32, kind="ExternalInput")
    xk = nc.dram_tensor("xk", [S, D], F32, kind="ExternalInput")
    xv = nc.dram_tensor("xv", [S, D], F32, kind="ExternalInput")
    ws = {
        n: nc.dram_tensor(n, [D, D], F32, kind="ExternalInput")
        for n in ("wq", "wk", "wv", "wg", "wo")
    }
    out = nc.dram_tensor("out", [S, D], F32, kind="ExternalOutput")

    with tile.TileContext(nc) as tc:
        if reps == 1:
            _body(tc, xq, xk, xv, ws, out, phases)
        else:
            with tc.For_i(0, reps, 1):
                _body(tc, xq, xk, xv, ws, out, phases)
    nc.compile()
    return nc


def _body(tc, xq, xk, xv, ws, out, phases=(1, 2)):
    nc = tc.nc
    from contextlib import ExitStack

    with ExitStack() as ctx:
        ep = ctx.enter_context

        dram = ep(tc.tile_pool(name="dram", bufs=1, space="DRAM"))
        qt_dram = dram.tile([P, DK, S], F32)       # Q^T scratch: [d, s]
        gt_dram = dram.tile([P, DK, S], BF16)      # gate^T scratch: [d, s]

        const = ep(tc.tile_pool(name="const", bufs=1))
        ident_f = const.tile([P, P], F32)
        make_identity(nc, ident_f)

        # ---- long-lived SBUF residents (96 KiB/part) ----
        kv_pool = ep(tc.tile_pool(name="kv", bufs=1))
        kT_sb = kv_pool.tile([P, DK, S], F32R)     # K^T  (64 KiB/part)
        v_sb = kv_pool.tile([P, ST, D], BF16)      # V natural (32 KiB/part)

        # =================== phase 1 ===================
        if 1 not in phases:
            nc.vector.memset(kT_sb, 0.0)
            nc.vector.memset(v_sb, 0.0)
        if 1 in phases:
          with tc.tile_pool(name="xslab", bufs=2) as slab_pool, \
               tc.tile_pool(name="xstage", bufs=4) as x_pool, \
               tc.tile_pool(name="wchunk", bufs=4) as w_pool, \
               tc.tile_pool(name="evict", bufs=3, space="PSUM") as evict_pool, \
               tc.tile_pool(name="tp", bufs=4, space="PSUM") as tp_pool:

              def load_w(name):
                  # weights ride the ACT HWDGE queue so Xk/Xq tile loads
                  # on the SP queue are never stuck behind a 4MB transfer;
                  # batched 4 chunks per DMA to keep sequencer issue cost low
                  tiles = []
                  for h in range(2):
                      wt = w_pool.tile([P, 4, D], F32R, tag="wchunk")
                      src = ws[name][h * 4 * P:(h + 1) * 4 * P, :].bitcast(F32R)
                      nc.scalar.dma_start(
                          out=wt, in_=src.rearrange("(c p) d -> p c d", p=P))
                      tiles.extend(wt[:, c, :] for c in range(4))
                  return tiles

              def make_slab(x_dram, sl):
                  """load TPS x-tiles of slab sl, PE-transpose into a slab
                  buffer [P, DK, SLW] (fp32r)"""
                  slab = slab_pool.tile([P, DK, SLW], F32R, tag="slab")
                  for st in range(TPS):
                      s = sl * TPS + st
                      xt = x_pool.tile([P, D], F32, tag="xstage")
                      nc.sync.dma_start(out=xt, in_=x_dram[s * P:(s + 1) * P, :])
                      for j in range(2):
                          pst = tp_pool.tile([P, 512], F32, tag="tp")
                          for i in range(4):
                              k = j * 4 + i
                              nc.tensor.transpose(
                                  pst[:, i * P:(i + 1) * P],
                                  xt[:, k * P:(k + 1) * P], ident_f)
                          dst = slab[:, j * 4:(j + 1) * 4, st * P:(st + 1) * P]
                          pr = pst.rearrange("p (a b) -> p a b", a=4)
                          nc.vector.tensor_copy(dst, pr)
                  return slab

              # -------- K section: K^T projection (fp32 resident) ---------
              wk = load_w("wk")
              wq = load_w("wq")      # prefetch under the K section
              slab = make_slab(xk, 0)
              for sl in range(NSL):
                  nslab = make_slab(xk, sl + 1) if sl + 1 < NSL \
                      else make_slab(xq, 0)
                  for m in range(DK):
                      ps = evict_pool.tile([P, 512], F32, tag="proj")
                      for k in range(DK):
                          _mm(nc, ps[:, :SLW], wk[k][:, m * P:(m + 1) * P],
                              slab[:, k, :], start=(k == 0), stop=(k == DK - 1))
                      nc.vector.tensor_copy(
                          kT_sb[:, m, sl * SLW:(sl + 1) * SLW], ps[:, :SLW])
                  slab = nslab

              # -------- Q+gate section (shared slabs) ---------------------
              wg = load_w("wg")
              # stage Wv's fp32 bits into the still-dead V buffer (ACT queue)
              for h in range(2):
                  dstv = v_sb[:, h * DK:(h + 1) * DK, :].bitcast(F32)
                  nc.scalar.dma_start(
                      out=dstv.rearrange("p (c two) b -> p c (two b)", two=2),
                      in_=ws["wv"][h * 4 * P:(h + 1) * 4 * P, :].rearrange(
                          "(c p) d -> p c d", p=P))

              with tc.tile_pool(name="qstage", bufs=1) as q_pool, \
                   tc.tile_pool(name="gstage", bufs=1) as g_pool:
                  for sl in range(NSL):
                      nslab = make_slab(xq, sl + 1) if sl + 1 < NSL else None
                      qstg = q_pool.tile([P, DK, SLW], F32, tag="qstage")
                      gstg = g_pool.tile([P, DK, SLW], BF16, tag="gstage")
                      for m in range(DK):
                          ps = evict_pool.tile([P, 512], F32, tag="proj")
                          for k in range(DK):
                              _mm(nc, ps[:, :SLW], wq[k][:, m * P:(m + 1) * P],
                                  slab[:, k, :], start=(k == 0),
                                  stop=(k == DK - 1))
                          nc.vector.tensor_copy(qstg[:, m, :], ps[:, :SLW])
                      # one batched store per slab keeps the ACT sequencer's
                      # ~0.6us per-DMA issue cost off the critical path
                      nc.scalar.dma_start(
                          out=qt_dram[:, :, sl * SLW:(sl + 1) * SLW], in_=qstg)
                      for m in range(DK):
                          ps = evict_pool.tile([P, 512], F32, tag="proj")
                          for k in range(DK):
                              _mm(nc, ps[:, :SLW], wg[k][:, m * P:(m + 1) * P],
                                  slab[:, k, :], start=(k == 0),
                                  stop=(k == DK - 1))
                          nc.scalar.activation(gstg[:, m, :], ps[:, :SLW],
                                               ACTF.Sigmoid)
                      nc.scalar.dma_start(
                          out=gt_dram[:, :, sl * SLW:(sl + 1) * SLW], in_=gstg)
                      slab = nslab

        # Wo bf16 resident: pool created after the K/QG pools close
        wo_pool = ep(tc.tile_pool(name="wo", bufs=1))
        wo_sb = wo_pool.tile([P, DK, D], BF16)     # 16 KiB/part
        if 1 not in phases:
            nc.vector.memset(wo_sb, 0.0)

        if 1 in phases:
          # -------- V section: bf16, DMA-xbar transposed input ----------
          with tc.tile_pool(name="wvbf", bufs=1) as wvbf_pool, \
               tc.tile_pool(name="wostage", bufs=1) as wo_stage, \
               tc.tile_pool(name="xvT", bufs=1) as xvT_pool, \
               tc.tile_pool(name="xvstage", bufs=3) as xv_pool, \
               tc.tile_pool(name="xvbf", bufs=3) as xvbf_pool, \
               tc.tile_pool(name="evict2", bufs=4, space="PSUM") as ev2_pool:
              xvT = xvT_pool.tile([P, DK, S], BF16)

              def xv_tile(s):
                  xt = xv_pool.tile([P, D], F32, tag="xvstage")
                  nc.sync.dma_start(out=xt, in_=xv[s * P:(s + 1) * P, :])
                  xb = xvbf_pool.tile([P, D], BF16, tag="xvbf")
                  nc.vector.tensor_copy(xb, xt)
                  # transpose waits on the cast; keep it off the SP queue so
                  # it cannot head-of-line block the xv tile loads
                  nc.scalar.dma_start(
                      out=xvT[:, :, s * P:(s + 1) * P], in_=xb, transpose=True)

              xv_tile(0)
              xv_tile(1)
              # cast the staged Wv bits (in v_sb) to bf16
              wv_bf = wvbf_pool.tile([P, DK, D], BF16)
              for k in range(DK):
                  nc.vector.tensor_copy(
                      wv_bf[:, k, :].rearrange("p (a b) -> p a b", a=2),
                      v_sb[:, 2 * k:2 * k + 2, :].bitcast(F32))
              for s in range(2, ST):
                  xv_tile(s)

              # Wo load + bf16 cast (ACT queue DMA, DVE cast)
              for h in range(2):
                  wt = wo_stage.tile([P, 4, D], F32, tag="wostage")
                  nc.scalar.dma_start(
                      out=wt, in_=ws["wo"][h * 4 * P:(h + 1) * 4 * P, :]
                      .rearrange("(c p) d -> p c d", p=P))
                  for c in range(4):
                      nc.vector.tensor_copy(wo_sb[:, h * 4 + c, :], wt[:, c, :])

              for s in range(ST):
                  pss = [ev2_pool.tile([P, 512], F32, tag="vproj",
                                       name=f"pss{_n}") for _n in range(2)]
                  for k in range(DK):
                      for n in range(2):
                          _mm(nc, pss[n], xvT[:, k, s * P:(s + 1) * P],
                              wv_bf[:, k, n * 512:(n + 1) * 512],
                              start=(k == 0), stop=(k == DK - 1))
                  for n in range(2):
                      nc.vector.tensor_copy(
                          v_sb[:, s, n * 512:(n + 1) * 512], pss[n])

        # =================== phase 2 ===================
        if 2 not in phases:
            return

        QB = 4                      # q tiles per block
        NBLK = ST // QB
        attnp = ep(tc.tile_pool(name="attnp", bufs=2))
        outp = ep(tc.tile_pool(name="outp", bufs=3))
        blkp = ep(tc.tile_pool(name="blkp", bufs=1))
        gtp = ep(tc.tile_pool(name="gtp", bufs=2))
        qtp = ep(tc.tile_pool(name="qtp", bufs=2))
        stats = ep(tc.tile_pool(name="stats", bufs=2 * QB + 2))
        ps_a = ep(tc.tile_pool(name="ps_a", bufs=5, space="PSUM"))
        ps_b = ep(tc.tile_pool(name="ps_b", bufs=3, space="PSUM"))

        qt_tiles = {}

        def load_qt(t):
            qt_sb = qtp.tile([P, DK, P], F32R, tag="qt", name="qt_sb")
            nc.scalar.dma_start(
                out=qt_sb, in_=qt_dram[:, :, t * P:(t + 1) * P].bitcast(F32R))
            qt_tiles[t] = qt_sb

        def head(t, attnT_blk, tq):
            """scores + per-bank softmax + DMA-transpose into attnT col tq"""
            # prefetch the NEXT tile's Q slice first: the dma issue must
            # precede this tile's exp instructions on the ACT queue, which
            # wait on scores and would head-of-line block the load
            if t + 1 < ST:
                load_qt(t + 1)
            qt_sb = qt_tiles.pop(t)

            negmax4 = stats.tile([P, NB], F32, tag="negmax4", name="negmax4")
            sums4 = stats.tile([P, NB], F32, tag="sums4", name="sums4")
            neg_max = stats.tile([P, 1], F32, tag="negmax", name="neg_max")
            c4 = stats.tile([P, NB], F32, tag="c4", name="c4")
            recip = stats.tile([P, 1], F32, tag="recip", name="recip")
            sumx = stats.tile([P, 1], F32, tag="sumx", name="sumx")

            score_ps = [ps_a.tile([P, 512], F32, tag="ps_a", name=f"sps{_n}")
                        for _n in range(NB)]
            for k in range(DK):
                for nb in range(NB):
                    _mm(nc, score_ps[nb], qt_sb[:, k, :],
                        kT_sb[:, k, nb * 512:(nb + 1) * 512],
                        start=(k == 0), stop=(k == DK - 1))
            attn = attnp.tile([P, S], BF16, tag="attn", name="attn")
            for nb in range(NB):
                nc.vector.tensor_reduce(
                    negmax4[:, nb:nb + 1], score_ps[nb], axis=AX.X,
                    op=ALU.max, negate=True)
                # exp with per-bank max: frees the psum bank without waiting
                # for the global row max
                nc.scalar.activation(
                    attn[:, nb * 512:(nb + 1) * 512], score_ps[nb], ACTF.Exp,
                    bias=negmax4[:, nb:nb + 1], accum_out=sums4[:, nb:nb + 1])
            # global max + per-bank correction c4 = exp(m_nb - M)
            nc.vector.tensor_reduce(neg_max, negmax4, axis=AX.X, op=ALU.min)
            nc.vector.tensor_scalar(
                out=c4, in0=negmax4, scalar1=neg_max, scalar2=None,
                op0=ALU.subtract)
            nc.scalar.activation(c4, c4, ACTF.Exp, scale=-1.0)
            nc.vector.tensor_tensor(out=sums4, in0=sums4, in1=c4, op=ALU.mult)
            nc.vector.tensor_reduce(sumx, sums4, axis=AX.X, op=ALU.add)
            nc.vector.reciprocal(recip, sumx)
            for nb in range(NB):
                nc.vector.tensor_scalar_mul(
                    attn[:, nb * 512:(nb + 1) * 512],
                    attn[:, nb * 512:(nb + 1) * 512], c4[:, nb:nb + 1])

            # one DMA xbar transpose: attnT_blk[p, kb, tq*P+j] = attn[j, kb*P+p]
            nc.sync.dma_start(
                out=attnT_blk[:, :, tq * P:(tq + 1) * P], in_=attn,
                transpose=True)
            return recip

        def tail_block(blk, attnT_blk, recips):
            q0 = blk * QB * P           # block q offset
            gt_sb = gtp.tile([P, DK, QB * P], BF16, tag="gt", name="gt_sb")
            nc.scalar.dma_start(out=gt_sb, in_=gt_dram[:, :, q0:q0 + QB * P])

            # ctx^T = V^T x attnT (bf16), evict fused with gate^T multiply
            ctxgT_blk = blkp.tile([P, DK, QB * P], BF16, name="ctxgT_blk")
            for mp in range(DK // 2):
                ps_c = [ps_b.tile([P, 512], F32, tag="ps_b", name=f"psc{_n}")
                        for _n in range(2)]
                for kb in range(ST):
                    for h in range(2):
                        m = mp * 2 + h
                        _mm(nc, ps_c[h], v_sb[:, kb, m * P:(m + 1) * P],
                            attnT_blk[:, kb, :],
                            start=(kb == 0), stop=(kb == ST - 1))
                for h in range(2):
                    m = mp * 2 + h
                    nc.vector.tensor_tensor(
                        out=ctxgT_blk[:, m, :], in0=ps_c[h],
                        in1=gt_sb[:, m, :], op=ALU.mult)

            # out = (ctxgT x Wo) * recip, per q tile
            for tq in range(QB):
                t = blk * QB + tq
                ps_o = [ps_b.tile([P, 512], F32, tag="ps_b", name=f"pso{_n}")
                        for _n in range(2)]
                for k in range(DK):
                    for n in range(2):
                        _mm(nc, ps_o[n],
                            ctxgT_blk[:, k, tq * P:(tq + 1) * P],
                            wo_sb[:, k, n * 512:(n + 1) * 512],
                            start=(k == 0), stop=(k == DK - 1))
                for n in range(2):
                    out_sb = outp.tile([P, 512], F32, tag="out", name="out_sb")
                    nc.vector.tensor_scalar_mul(out_sb, ps_o[n], recips[tq])
                    nc.sync.dma_start(
                        out=out[t * P:(t + 1) * P, n * 512:(n + 1) * 512],
                        in_=out_sb)

        prev = None
        load_qt(0)
        for blk in range(NBLK):
            attnT_blk = blkp.tile([P, ST, QB * P], BF16, name="attnT_blk",
                                  tag=f"attnT{blk % 2}")
            recips = []
            for tq in range(QB):
                recips.append(head(blk * QB + tq, attnT_blk, tq))
            if prev is not None:
                tail_block(*prev)
            prev = (blk, attnT_blk, recips)
        tail_block(*prev)


_CACHE = {}


def _get_program(zero_bias: bool):
    if zero_bias not in _CACHE:
        _CACHE[zero_bias] = build_program(zero_bias)
    return _CACHE[zero_bias]


def kernel(queries, keys, values, Wq, bq, Wk, bk, Wv, bv, Wg, bg, Wo, bo):
    queries = np.ascontiguousarray(np.asarray(queries, dtype=np.float32))
    keys = np.ascontiguousarray(np.asarray(keys, dtype=np.float32))
    values = np.ascontiguousarray(np.asarray(values, dtype=np.float32))
    wdict = {
        "wq": np.ascontiguousarray(np.asarray(Wq, np.float32)),
        "wk": np.ascontiguousarray(np.asarray(Wk, np.float32)),
        "wv": np.ascontiguousarray(np.asarray(Wv, np.float32)),
        "wg": np.ascontiguousarray(np.asarray(Wg, np.float32)),
        "wo": np.ascontiguousarray(np.asarray(Wo, np.float32)),
    }
    bdict = {
        "bq": np.ascontiguousarray(np.asarray(bq, np.float32)),
        "bk": np.ascontiguousarray(np.asarray(bk, np.float32)),
        "bv": np.ascontiguousarray(np.asarray(bv, np.float32)),
        "bg": np.ascontiguousarray(np.asarray(bg, np.float32)),
        "bo": np.ascontiguousarray(np.asarray(bo, np.float32)),
    }
    zero_bias = all(not np.any(v) for v in bdict.values())
    if not zero_bias:
        # Bias-enabled device path is not wired up; the problem's
        # setup_inputs() uses all-zero biases, so this branch only exists
        # for off-spec inputs. Compute on host for correctness.
        return _host_reference(queries, keys, values, wdict, bdict)
    nc = _get_program(True)

    in_maps = []
    for b in range(B):
        m = {"xq": queries[b], "xk": keys[b], "xv": values[b]}
        m.update(wdict)
        in_maps.append(m)
    res = run_bass_kernel_spmd(nc, in_maps, core_ids=list(range(B)))
    return np.stack([res.results[b]["out"] for b in range(B)], axis=0)


def _host_reference(queries, keys, values, w, bdict):
    out = np.empty_like(queries)
    for b in range(B):
        q = queries[b] @ w["wq"] + bdict["bq"]
        k = keys[b] @ w["wk"] + bdict["bk"]
        v = values[b] @ w["wv"] + bdict["bv"]
        s = q @ k.T
        s -= s.max(axis=-1, keepdims=True)
        e = np.exp(s)
        a = e / e.sum(axis=-1, keepdims=True)
        gate = 1.0 / (1.0 + np.exp(-(queries[b] @ w["wg"] + bdict["bg"])))
        out[b] = ((a @ v) * gate) @ w["wo"] + bdict["bo"]
    return out


# revision 19
# speedup vs baseline: 1.0651x; 1.0388x over previous
"""Gated attention layer (B=8, S=2048, D=1024) on 8 Trainium2 NeuronCores.

Sharding: data-parallel over batch B — core b computes batch element b
end-to-end (weights replicated). No collectives.

Per-core dataflow:
  phase 1 (slab-structured, order K -> Q+gate -> V):
    K section: per 256-col slab of Xk^T: PE-transpose 2 x-tiles into the
      slab buffer (fp32), then K^T slab = Wk^T x slab (fp32r), evicted
      into the SBUF-resident K^T.  Transposes for slab i+1 are emitted
      before the projection of slab i so PE never starves.  Wk and Wq
      prefetch on the ACT HWDGE queue while Xk streams on the SP queue.
    Q+gate section: slabs of Xq^T feed both Q^T (fp32 -> DRAM scratch)
      and gate^T = sigmoid(Wg^T Xq^T) (bf16 -> DRAM scratch).  Wg loads
      at section start; Wv's fp32 bits stage into the still-dead V
      buffer so the V section can start without a weight-load bubble.
    V section: Xv tiles are cast to bf16 and DMA-xbar-transposed into
      Xv^T (no PE work), then V = Xv Wv in natural layout (bf16).
      Wo loads/casts here and phase 2's first Q^T tile prefetches under
      this section's matmuls.
  phase 2: blocks of 4 q tiles.  Per q tile: scores = Q^T slice x K^T
      (fp32r, PSUM), softmax along the free axis (DVE per-bank
      max-reduce + ACT exp with fused bias/row-sum, then a cheap
      exp(m_nb - M) cross-bank correction so score banks free early),
      then ONE DMA-xbar transpose of the bf16 attention tile into the
      block buffer (no PE transposes).  Per block: ctx^T = V^T x attnT
      (bf16) with the gate^T multiply fused into eviction, then
      out = ctxgT x Wo (bf16) with the 1/sum normalization fused into
      the final eviction.  Head of block b+1 is emitted before the
      tail of block b so PE never waits on the softmax chain.
"""

import numpy as np

import concourse.bass as bass
import concourse.tile as tile
from concourse import bacc, mybir
from concourse.bass_utils import run_bass_kernel_spmd
from concourse.masks import make_identity

B, S, D = 8, 2048, 1024
P = 128
DK = D // P      # 8 contraction chunks of 128
ST = S // P      # 16 seq tiles of 128
NB = S // 512    # 4 scores banks of 512
SLW = 256        # slab width (columns of X^T per slab)
NSL = S // SLW   # 8 slabs per input
TPS = SLW // P   # 2 s-tiles per slab

F32 = mybir.dt.float32
F32R = mybir.dt.float32r
BF16 = mybir.dt.bfloat16
AX = mybir.AxisListType
ALU = mybir.AluOpType
ACTF = mybir.ActivationFunctionType


def _mm(nc, out, lhsT, rhs, start, stop):
    nc.tensor.matmul(out, lhsT, rhs, start=start, stop=stop)


def build_program(zero_bias: bool, debug: bool = False, reps: int = 1, phases=(1, 2)):
    assert zero_bias, "device path supports zero biases only"
    nc = bacc.Bacc(None, target_bir_lowering=False, debug=debug)

    xq = nc.dram_tensor("xq", [S, D], F32, kind="ExternalInput")
    xk = nc.dram_tensor("xk", [S, D], F32, kind="ExternalInput")
    xv = nc.dram_tensor("xv", [S, D], F32, kind="ExternalInput")
    ws = {
        n: nc.dram_tensor(n, [D, D], F32, kind="ExternalInput")
        for n in ("wq", "wk", "wv", "wg", "wo")
    }
    out = nc.dram_tensor("out", [S, D], F32, kind="ExternalOutput")

    with tile.TileContext(nc) as tc:
        if reps == 1:
            _body(tc, xq, xk, xv, ws, out, phases)
        else:
            with tc.For_i(0, reps, 1):
                _body(tc, xq, xk, xv, ws, out, phases)
    nc.compile()
    return nc


def _body(tc, xq, xk, xv, ws, out, phases=(1, 2)):
    nc = tc.nc
    from contextlib import ExitStack

    with ExitStack() as ctx:
        ep = ctx.enter_context

        dram = ep(tc.tile_pool(name="dram", bufs=1, space="DRAM"))
        qt_dram = dram.tile([P, DK, S], F32)       # Q^T scratch: [d, s]
        gt_dram = dram.tile([P, DK, S], BF16)      # gate^T scratch: [d, s]

        const = ep(tc.tile_pool(name="const", bufs=1))
        ident_f = const.tile([P, P], F32)
        make_identity(nc, ident_f)

        # ---- long-lived SBUF residents (96 KiB/part) ----
        kv_pool = ep(tc.tile_pool(name="kv", bufs=1))
        kT_sb = kv_pool.tile([P, DK, S], F32R)     # K^T  (64 KiB/part)
        v_sb = kv_pool.tile([P, ST, D], BF16)      # V natural (32 KiB/part)

        # =================== phase 1 ===================
        if 1 not in phases:
            nc.vector.memset(kT_sb, 0.0)
            nc.vector.memset(v_sb, 0.0)
        if 1 in phases:
          with tc.tile_pool(name="xslab", bufs=2) as slab_pool, \
               tc.tile_pool(name="xstage", bufs=4) as x_pool, \
               tc.tile_pool(name="wchunk", bufs=4) as w_pool, \
               tc.tile_pool(name="evict", bufs=3, space="PSUM") as evict_pool, \
               tc.tile_pool(name="tp", bufs=4, space="PSUM") as tp_pool:

              def load_w(name):
                  # weights ride the ACT HWDGE queue so Xk/Xq tile loads
                  # on the SP queue are never stuck behind a 4MB transfer;
                  # batched 4 chunks per DMA to keep sequencer issue cost low
                  tiles = []
                  for h in range(2):
                      wt = w_pool.tile([P, 4, D], F32R, tag="wchunk")
                      src = ws[name][h * 4 * P:(h + 1) * 4 * P, :].bitcast(F32R)
                      nc.scalar.dma_start(
                          out=wt, in_=src.rearrange("(c p) d -> p c d", p=P))
                      tiles.extend(wt[:, c, :] for c in range(4))
                  return tiles

              def make_slab(x_dram, sl):
                  """load TPS x-tiles of slab sl, PE-transpose into a slab
                  buffer [P, DK, SLW] (fp32r)"""
                  slab = slab_pool.tile([P, DK, SLW], F32R, tag="slab")
                  for st in range(TPS):
                      s = sl * TPS + st
                      xt = x_pool.tile([P, D], F32, tag="xstage")
                      nc.sync.dma_start(out=xt, in_=x_dram[s * P:(s + 1) * P, :])
                      for j in range(2):
                          pst = tp_pool.tile([P, 512], F32, tag="tp")
                          for i in range(4):
                              k = j * 4 + i
                              nc.tensor.transpose(
                                  pst[:, i * P:(i + 1) * P],
                                  xt[:, k * P:(k + 1) * P], ident_f)
                          dst = slab[:, j * 4:(j + 1) * 4, st * P:(st + 1) * P]
                          pr = pst.rearrange("p (a b) -> p a b", a=4)
                          nc.vector.tensor_copy(dst, pr)
                  return slab

              # -------- K section: K^T projection (fp32 resident) ---------
              wk = load_w("wk")
              wq = load_w("wq")      # prefetch under the K section
              slab = make_slab(xk, 0)
              for sl in range(NSL):
                  nslab = make_slab(xk, sl + 1) if sl + 1 < NSL \
                      else make_slab(xq, 0)
                  for m in range(DK):
                      ps = evict_pool.tile([P, 512], F32, tag="proj")
                      for k in range(DK):
                          _mm(nc, ps[:, :SLW], wk[k][:, m * P:(m + 1) * P],
                              slab[:, k, :], start=(k == 0), stop=(k == DK - 1))
                      nc.vector.tensor_copy(
                          kT_sb[:, m, sl * SLW:(sl + 1) * SLW], ps[:, :SLW])
                  slab = nslab

              # -------- Q+gate section (shared slabs) ---------------------
              wg = load_w("wg")
              # stage Wv's fp32 bits into the still-dead V buffer (ACT queue)
              for h in range(2):
                  dstv = v_sb[:, h * DK:(h + 1) * DK, :].bitcast(F32)
                  nc.scalar.dma_start(
                      out=dstv.rearrange("p (c two) b -> p c (two b)", two=2),
                      in_=ws["wv"][h * 4 * P:(h + 1) * 4 * P, :].rearrange(
                          "(c p) d -> p c d", p=P))

              with tc.tile_pool(name="qstage", bufs=1) as q_pool, \
                   tc.tile_pool(name="gstage", bufs=1) as g_pool:
                  for sl in range(NSL):
                      nslab = make_slab(xq, sl + 1) if sl + 1 < NSL else None
                      qstg = q_pool.tile([P, DK, SLW], F32, tag="qstage")
                      gstg = g_pool.tile([P, DK, SLW], BF16, tag="gstage")
                      for m in range(DK):
                          ps = evict_pool.tile([P, 512], F32, tag="proj")
                          for k in range(DK):
                              _mm(nc, ps[:, :SLW], wq[k][:, m * P:(m + 1) * P],
                                  slab[:, k, :], start=(k == 0),
                                  stop=(k == DK - 1))
                          nc.vector.tensor_copy(qstg[:, m, :], ps[:, :SLW])
                      # one batched store per slab keeps the ACT sequencer's
                      # ~0.6us per-DMA issue cost off the critical path
                      nc.scalar.dma_start(
                          out=qt_dram[:, :, sl * SLW:(sl + 1) * SLW], in_=qstg)
                      for m in range(DK):
                          ps = evict_pool.tile([P, 512], F32, tag="proj")
                          for k in range(DK):
                              _mm(nc, ps[:, :SLW], wg[k][:, m * P:(m + 1) * P],
                                  slab[:, k, :], start=(k == 0),
                                  stop=(k == DK - 1))
                          nc.scalar.activation(gstg[:, m, :], ps[:, :SLW],
                                               ACTF.Sigmoid)
                      nc.scalar.dma_start(
                          out=gt_dram[:, :, sl * SLW:(sl + 1) * SLW], in_=gstg)
                      slab = nslab

        # Wo bf16 resident: pool created after the K/QG pools close
        wo_pool = ep(tc.tile_pool(name="wo", bufs=1))
        wo_sb = wo_pool.tile([P, DK, D], BF16)     # 16 KiB/part
        if 1 not in phases:
            nc.vector.memset(wo_sb, 0.0)

        if 1 in phases:
          # -------- V section: bf16, DMA-xbar transposed input ----------
          with tc.tile_pool(name="wvbf", bufs=1) as wvbf_pool, \
               tc.tile_pool(name="wostage", bufs=1) as wo_stage, \
               tc.tile_pool(name="xvT", bufs=1) as xvT_pool, \
               tc.tile_pool(name="xvstage", bufs=4) as xv_pool, \
               tc.tile_pool(name="xvbf", bufs=4) as xvbf_pool, \
               tc.tile_pool(name="evict2", bufs=4, space="PSUM") as ev2_pool:
              xvT = xvT_pool.tile([P, DK, S], BF16)

              def xv_tile(s):
                  xt = xv_pool.tile([P, D], F32, tag="xvstage")
                  nc.sync.dma_start(out=xt, in_=xv[s * P:(s + 1) * P, :])
                  xb = xvbf_pool.tile([P, D], BF16, tag="xvbf")
                  nc.vector.tensor_copy(xb, xt)
                  # transpose waits on the cast; keep it off the SP queue so
                  # it cannot head-of-line block the xv tile loads
                  nc.scalar.dma_start(
                      out=xvT[:, :, s * P:(s + 1) * P], in_=xb, transpose=True)

              xv_tile(0)
              xv_tile(1)
              # cast the staged Wv bits (in v_sb) to bf16
              wv_bf = wvbf_pool.tile([P, DK, D], BF16)
              for k in range(DK):
                  nc.vector.tensor_copy(
                      wv_bf[:, k, :].rearrange("p (a b) -> p a b", a=2),
                      v_sb[:, 2 * k:2 * k + 2, :].bitcast(F32))
              for s in range(2, ST):
                  xv_tile(s)

              for s in range(ST):
                  pss = [ev2_pool.tile([P, 512], F32, tag="vproj",
                                       name=f"pss{_n}") for _n in range(2)]
                  for k in range(DK):
                      for n in range(2):
                          _mm(nc, pss[n], xvT[:, k, s * P:(s + 1) * P],
                              wv_bf[:, k, n * 512:(n + 1) * 512],
                              start=(k == 0), stop=(k == DK - 1))
                  for n in range(2):
                      nc.vector.tensor_copy(
                          v_sb[:, s, n * 512:(n + 1) * 512], pss[n])

              # Wo load + bf16 cast, emitted last: needed only in phase 2
              for h in range(2):
                  wt = wo_stage.tile([P, 4, D], F32, tag="wostage")
                  nc.scalar.dma_start(
                      out=wt, in_=ws["wo"][h * 4 * P:(h + 1) * 4 * P, :]
                      .rearrange("(c p) d -> p c d", p=P))
                  for c in range(4):
                      nc.vector.tensor_copy(wo_sb[:, h * 4 + c, :], wt[:, c, :])

        # =================== phase 2 ===================
        if 2 not in phases:
            return

        QB = 4                      # q tiles per block
        NBLK = ST // QB
        attnp = ep(tc.tile_pool(name="attnp", bufs=2))
        outp = ep(tc.tile_pool(name="outp", bufs=3))
        blkp = ep(tc.tile_pool(name="blkp", bufs=1))
        gtp = ep(tc.tile_pool(name="gtp", bufs=2))
        qtp = ep(tc.tile_pool(name="qtp", bufs=2))
        stats = ep(tc.tile_pool(name="stats", bufs=2 * QB + 2))
        ps_a = ep(tc.tile_pool(name="ps_a", bufs=5, space="PSUM"))
        ps_b = ep(tc.tile_pool(name="ps_b", bufs=3, space="PSUM"))

        qt_tiles = {}

        def load_qt(t):
            qt_sb = qtp.tile([P, DK, P], F32R, tag="qt", name="qt_sb")
            nc.scalar.dma_start(
                out=qt_sb, in_=qt_dram[:, :, t * P:(t + 1) * P].bitcast(F32R))
            qt_tiles[t] = qt_sb

        def head(t, attnT_blk, tq):
            """scores + per-bank softmax + DMA-transpose into attnT col tq"""
            # prefetch the NEXT tile's Q slice first: the dma issue must
            # precede this tile's exp instructions on the ACT queue, which
            # wait on scores and would head-of-line block the load
            if t + 1 < ST:
                load_qt(t + 1)
            qt_sb = qt_tiles.pop(t)

            negmax4 = stats.tile([P, NB], F32, tag="negmax4", name="negmax4")
            sums4 = stats.tile([P, NB], F32, tag="sums4", name="sums4")
            neg_max = stats.tile([P, 1], F32, tag="negmax", name="neg_max")
            c4 = stats.tile([P, NB], F32, tag="c4", name="c4")
            recip = stats.tile([P, 1], F32, tag="recip", name="recip")
            sumx = stats.tile([P, 1], F32, tag="sumx", name="sumx")

            score_ps = [ps_a.tile([P, 512], F32, tag="ps_a", name=f"sps{_n}")
                        for _n in range(NB)]
            attn = attnp.tile([P, S], BF16, tag="attn", name="attn")
            # nb-major: each bank completes after 8 MMs and its softmax evict
            # starts while later banks still stream, so psum banks recycle
            # pipelined instead of all-at-once at tile end
            for nb in range(NB):
                for k in range(DK):
                    _mm(nc, score_ps[nb], qt_sb[:, k, :],
                        kT_sb[:, k, nb * 512:(nb + 1) * 512],
                        start=(k == 0), stop=(k == DK - 1))
                nc.vector.tensor_reduce(
                    negmax4[:, nb:nb + 1], score_ps[nb], axis=AX.X,
                    op=ALU.max, negate=True)
                # exp with per-bank max: frees the psum bank without waiting
                # for the global row max
                nc.scalar.activation(
                    attn[:, nb * 512:(nb + 1) * 512], score_ps[nb], ACTF.Exp,
                    bias=negmax4[:, nb:nb + 1], accum_out=sums4[:, nb:nb + 1])
            # global max + per-bank correction c4 = exp(m_nb - M)
            nc.vector.tensor_reduce(neg_max, negmax4, axis=AX.X, op=ALU.min)
            nc.vector.tensor_scalar(
                out=c4, in0=negmax4, scalar1=neg_max, scalar2=None,
                op0=ALU.subtract)
            nc.scalar.activation(c4, c4, ACTF.Exp, scale=-1.0)
            nc.vector.tensor_tensor(out=sums4, in0=sums4, in1=c4, op=ALU.mult)
            nc.vector.tensor_reduce(sumx, sums4, axis=AX.X, op=ALU.add)
            nc.vector.reciprocal(recip, sumx)
            for nb in range(NB):
                nc.vector.tensor_scalar_mul(
                    attn[:, nb * 512:(nb + 1) * 512],
                    attn[:, nb * 512:(nb + 1) * 512], c4[:, nb:nb + 1])

            # one DMA xbar transpose: attnT_blk[p, kb, tq*P+j] = attn[j, kb*P+p]
            nc.sync.dma_start(
                out=attnT_blk[:, :, tq * P:(tq + 1) * P], in_=attn,
                transpose=True)
            return recip

        def tail_block(blk, attnT_blk, recips):
            q0 = blk * QB * P           # block q offset
            gt_sb = gtp.tile([P, DK, QB * P], BF16, tag="gt", name="gt_sb")
            nc.scalar.dma_start(out=gt_sb, in_=gt_dram[:, :, q0:q0 + QB * P])

            # ctx^T = V^T x attnT (bf16), evict fused with gate^T multiply
            ctxgT_blk = blkp.tile([P, DK, QB * P], BF16, name="ctxgT_blk")
            for mp in range(DK // 2):
                ps_c = [ps_b.tile([P, 512], F32, tag="ps_b", name=f"psc{_n}")
                        for _n in range(2)]
                for kb in range(ST):
                    for h in range(2):
                        m = mp * 2 + h
                        _mm(nc, ps_c[h], v_sb[:, kb, m * P:(m + 1) * P],
                            attnT_blk[:, kb, :],
                            start=(kb == 0), stop=(kb == ST - 1))
                for h in range(2):
                    m = mp * 2 + h
                    nc.vector.tensor_tensor(
                        out=ctxgT_blk[:, m, :], in0=ps_c[h],
                        in1=gt_sb[:, m, :], op=ALU.mult)

            # out = (ctxgT x Wo) * recip, per q tile
            for tq in range(QB):
                t = blk * QB + tq
                ps_o = [ps_b.tile([P, 512], F32, tag="ps_b", name=f"pso{_n}")
                        for _n in range(2)]
                for k in range(DK):
                    for n in range(2):
                        _mm(nc, ps_o[n],
                            ctxgT_blk[:, k, tq * P:(tq + 1) * P],
                            wo_sb[:, k, n * 512:(n + 1) * 512],
                            start=(k == 0), stop=(k == DK - 1))
                for n in range(2):
                    out_sb = outp.tile([P, 512], F32, tag="out", name="out_sb")
                    nc.vector.tensor_scalar_mul(out_sb, ps_o[n], recips[tq])
                    nc.sync.dma_start(
                        out=out[t * P:(t + 1) * P, n * 512:(n + 1) * 512],
                        in_=out_sb)

        prev = None
        load_qt(0)
        for blk in range(NBLK):
            attnT_blk = blkp.tile([P, ST, QB * P], BF16, name="attnT_blk",
                                  tag=f"attnT{blk % 2}")
            recips = []
            for tq in range(QB):
                recips.append(head(blk * QB + tq, attnT_blk, tq))
            if prev is not None:
                tail_block(*prev)
            prev = (blk, attnT_blk, recips)
        tail_block(*prev)


_CACHE = {}


def _get_program(zero_bias: bool):
    if zero_bias not in _CACHE:
        _CACHE[zero_bias] = build_program(zero_bias)
    return _CACHE[zero_bias]


def kernel(queries, keys, values, Wq, bq, Wk, bk, Wv, bv, Wg, bg, Wo, bo):
    queries = np.ascontiguousarray(np.asarray(queries, dtype=np.float32))
    keys = np.ascontiguousarray(np.asarray(keys, dtype=np.float32))
    values = np.ascontiguousarray(np.asarray(values, dtype=np.float32))
    wdict = {
        "wq": np.ascontiguousarray(np.asarray(Wq, np.float32)),
        "wk": np.ascontiguousarray(np.asarray(Wk, np.float32)),
        "wv": np.ascontiguousarray(np.asarray(Wv, np.float32)),
        "wg": np.ascontiguousarray(np.asarray(Wg, np.float32)),
        "wo": np.ascontiguousarray(np.asarray(Wo, np.float32)),
    }
    bdict = {
        "bq": np.ascontiguousarray(np.asarray(bq, np.float32)),
        "bk": np.ascontiguousarray(np.asarray(bk, np.float32)),
        "bv": np.ascontiguousarray(np.asarray(bv, np.float32)),
        "bg": np.ascontiguousarray(np.asarray(bg, np.float32)),
        "bo": np.ascontiguousarray(np.asarray(bo, np.float32)),
    }
    zero_bias = all(not np.any(v) for v in bdict.values())
    if not zero_bias:
        # Bias-enabled device path is not wired up; the problem's
        # setup_inputs() uses all-zero biases, so this branch only exists
        # for off-spec inputs. Compute on host for correctness.
        return _host_reference(queries, keys, values, wdict, bdict)
    nc = _get_program(True)

    in_maps = []
    for b in range(B):
        m = {"xq": queries[b], "xk": keys[b], "xv": values[b]}
        m.update(wdict)
        in_maps.append(m)
    res = run_bass_kernel_spmd(nc, in_maps, core_ids=list(range(B)))
    return np.stack([res.results[b]["out"] for b in range(B)], axis=0)


def _host_reference(queries, keys, values, w, bdict):
    out = np.empty_like(queries)
    for b in range(B):
        q = queries[b] @ w["wq"] + bdict["bq"]
        k = keys[b] @ w["wk"] + bdict["bk"]
        v = values[b] @ w["wv"] + bdict["bv"]
        s = q @ k.T
        s -= s.max(axis=-1, keepdims=True)
        e = np.exp(s)
        a = e / e.sum(axis=-1, keepdims=True)
        gate = 1.0 / (1.0 + np.exp(-(queries[b] @ w["wg"] + bdict["bg"])))
        out[b] = ((a @ v) * gate) @ w["wo"] + bdict["bo"]
    return out


# revision 23
# speedup vs baseline: 1.0872x; 1.0207x over previous
"""Gated attention layer (B=8, S=2048, D=1024) on 8 Trainium2 NeuronCores.

Sharding: data-parallel over batch B — core b computes batch element b
end-to-end (weights replicated). No collectives.

Per-core dataflow:
  phase 1 (slab-structured, order K -> Q+gate -> V):
    K section: per 256-col slab of Xk^T: PE-transpose 2 x-tiles into the
      slab buffer (fp32), then K^T slab = Wk^T x slab (fp32r), evicted
      into the SBUF-resident K^T.  Transposes for slab i+1 are emitted
      before the projection of slab i so PE never starves.  Wk and Wq
      prefetch on the ACT HWDGE queue while Xk streams on the SP queue.
    Q+gate section: slabs of Xq^T feed both Q^T (fp32 -> DRAM scratch)
      and gate^T = sigmoid(Wg^T Xq^T) (bf16 -> DRAM scratch).  Wg loads
      at section start; Wv's fp32 bits stage into the still-dead V
      buffer so the V section can start without a weight-load bubble.
    V section: Xv tiles are cast to bf16 and DMA-xbar-transposed into
      Xv^T (no PE work), then V = Xv Wv in natural layout (bf16).
      Wo loads/casts here and phase 2's first Q^T tile prefetches under
      this section's matmuls.
  phase 2: blocks of 4 q tiles.  Per q tile: scores = Q^T slice x K^T
      (fp32r, PSUM), softmax along the free axis (DVE per-bank
      max-reduce + ACT exp with fused bias/row-sum, then a cheap
      exp(m_nb - M) cross-bank correction so score banks free early),
      then ONE DMA-xbar transpose of the bf16 attention tile into the
      block buffer (no PE transposes).  Per block: ctx^T = V^T x attnT
      (bf16) with the gate^T multiply fused into eviction, then
      out = ctxgT x Wo (bf16) with the 1/sum normalization fused into
      the final eviction.  Head of block b+1 is emitted before the
      tail of block b so PE never waits on the softmax chain.
"""

import numpy as np

import concourse.bass as bass
import concourse.tile as tile
from concourse import bacc, mybir
from concourse.bass_utils import run_bass_kernel_spmd
from concourse.masks import make_identity

B, S, D = 8, 2048, 1024
P = 128
DK = D // P      # 8 contraction chunks of 128
ST = S // P      # 16 seq tiles of 128
NB = S // 512    # 4 scores banks of 512
SLW = 256        # slab width (columns of X^T per slab)
NSL = S // SLW   # 8 slabs per input
TPS = SLW // P   # 2 s-tiles per slab

F32 = mybir.dt.float32
F32R = mybir.dt.float32r
BF16 = mybir.dt.bfloat16
AX = mybir.AxisListType
ALU = mybir.AluOpType
ACTF = mybir.ActivationFunctionType


def _mm(nc, out, lhsT, rhs, start, stop):
    nc.tensor.matmul(out, lhsT, rhs, start=start, stop=stop)


def build_program(zero_bias: bool, debug: bool = False, reps: int = 1, phases=(1, 2)):
    assert zero_bias, "device path supports zero biases only"
    nc = bacc.Bacc(None, target_bir_lowering=False, debug=debug)

    xq = nc.dram_tensor("xq", [S, D], F32, kind="ExternalInput")
    xk = nc.dram_tensor("xk", [S, D], F32, kind="ExternalInput")
    xv = nc.dram_tensor("xv", [S, D], F32, kind="ExternalInput")
    ws = {
        n: nc.dram_tensor(n, [D, D], F32, kind="ExternalInput")
        for n in ("wq", "wk", "wv", "wg", "wo")
    }
    out = nc.dram_tensor("out", [S, D], F32, kind="ExternalOutput")

    with tile.TileContext(nc) as tc:
        if reps == 1:
            _body(tc, xq, xk, xv, ws, out, phases)
        else:
            with tc.For_i(0, reps, 1):
                _body(tc, xq, xk, xv, ws, out, phases)
    nc.compile()
    return nc


def _body(tc, xq, xk, xv, ws, out, phases=(1, 2)):
    nc = tc.nc
    from contextlib import ExitStack

    with ExitStack() as ctx:
        ep = ctx.enter_context

        dram = ep(tc.tile_pool(name="dram", bufs=1, space="DRAM"))
        qt_dram = dram.tile([P, DK, S], F32)       # Q^T scratch: [d, s]
        gt_dram = dram.tile([P, DK, S], BF16)      # gate^T scratch: [d, s]

        const = ep(tc.tile_pool(name="const", bufs=1))
        ident_f = const.tile([P, P], F32)
        make_identity(nc, ident_f)

        # ---- long-lived SBUF residents (96 KiB/part) ----
        kv_pool = ep(tc.tile_pool(name="kv", bufs=1))
        kT_sb = kv_pool.tile([P, DK, S], F32R)     # K^T  (64 KiB/part)
        v_sb = kv_pool.tile([P, ST, D], BF16)      # V natural (32 KiB/part)

        # =================== phase 1 ===================
        if 1 not in phases:
            nc.vector.memset(kT_sb, 0.0)
            nc.vector.memset(v_sb, 0.0)
        if 1 in phases:
          with tc.tile_pool(name="xslab", bufs=2) as slab_pool, \
               tc.tile_pool(name="xstage", bufs=4) as x_pool, \
               tc.tile_pool(name="wchunk", bufs=4) as w_pool, \
               tc.tile_pool(name="evict", bufs=3, space="PSUM") as evict_pool, \
               tc.tile_pool(name="tp", bufs=4, space="PSUM") as tp_pool:

              def load_w(name):
                  # weights ride the ACT HWDGE queue so Xk/Xq tile loads
                  # on the SP queue are never stuck behind a 4MB transfer;
                  # batched 4 chunks per DMA to keep sequencer issue cost low
                  tiles = []
                  for h in range(2):
                      wt = w_pool.tile([P, 4, D], F32R, tag="wchunk")
                      src = ws[name][h * 4 * P:(h + 1) * 4 * P, :].bitcast(F32R)
                      nc.scalar.dma_start(
                          out=wt, in_=src.rearrange("(c p) d -> p c d", p=P))
                      tiles.extend(wt[:, c, :] for c in range(4))
                  return tiles

              def make_slab(x_dram, sl):
                  """load TPS x-tiles of slab sl, PE-transpose into a slab
                  buffer [P, DK, SLW] (fp32r)"""
                  slab = slab_pool.tile([P, DK, SLW], F32R, tag="slab")
                  for st in range(TPS):
                      s = sl * TPS + st
                      xt = x_pool.tile([P, D], F32, tag="xstage")
                      nc.sync.dma_start(out=xt, in_=x_dram[s * P:(s + 1) * P, :])
                      for j in range(2):
                          pst = tp_pool.tile([P, 512], F32, tag="tp")
                          for i in range(4):
                              k = j * 4 + i
                              nc.tensor.transpose(
                                  pst[:, i * P:(i + 1) * P],
                                  xt[:, k * P:(k + 1) * P], ident_f)
                          dst = slab[:, j * 4:(j + 1) * 4, st * P:(st + 1) * P]
                          pr = pst.rearrange("p (a b) -> p a b", a=4)
                          nc.vector.tensor_copy(dst, pr)
                  return slab

              # -------- K section: K^T projection (fp32 resident) ---------
              wk = load_w("wk")
              wq = load_w("wq")      # prefetch under the K section
              slab = make_slab(xk, 0)
              for sl in range(NSL):
                  nslab = make_slab(xk, sl + 1) if sl + 1 < NSL \
                      else make_slab(xq, 0)
                  for m in range(DK):
                      ps = evict_pool.tile([P, 512], F32, tag="proj")
                      for k in range(DK):
                          _mm(nc, ps[:, :SLW], wk[k][:, m * P:(m + 1) * P],
                              slab[:, k, :], start=(k == 0), stop=(k == DK - 1))
                      nc.vector.tensor_copy(
                          kT_sb[:, m, sl * SLW:(sl + 1) * SLW], ps[:, :SLW])
                  slab = nslab

              # -------- Q+gate section (shared slabs) ---------------------
              wg = load_w("wg")
              # stage Wv's fp32 bits into the still-dead V buffer (ACT queue)
              for h in range(2):
                  dstv = v_sb[:, h * DK:(h + 1) * DK, :].bitcast(F32)
                  nc.scalar.dma_start(
                      out=dstv.rearrange("p (c two) b -> p c (two b)", two=2),
                      in_=ws["wv"][h * 4 * P:(h + 1) * 4 * P, :].rearrange(
                          "(c p) d -> p c d", p=P))

              with tc.tile_pool(name="qstage", bufs=1) as q_pool, \
                   tc.tile_pool(name="gstage", bufs=1) as g_pool:
                  for sl in range(NSL):
                      nslab = make_slab(xq, sl + 1) if sl + 1 < NSL else None
                      qstg = q_pool.tile([P, DK, SLW], F32, tag="qstage")
                      gstg = g_pool.tile([P, DK, SLW], BF16, tag="gstage")
                      for m in range(DK):
                          ps = evict_pool.tile([P, 512], F32, tag="proj")
                          for k in range(DK):
                              _mm(nc, ps[:, :SLW], wq[k][:, m * P:(m + 1) * P],
                                  slab[:, k, :], start=(k == 0),
                                  stop=(k == DK - 1))
                          nc.vector.tensor_copy(qstg[:, m, :], ps[:, :SLW])
                      # one batched store per slab keeps the ACT sequencer's
                      # ~0.6us per-DMA issue cost off the critical path
                      nc.scalar.dma_start(
                          out=qt_dram[:, :, sl * SLW:(sl + 1) * SLW], in_=qstg)
                      for m in range(DK):
                          ps = evict_pool.tile([P, 512], F32, tag="proj")
                          for k in range(DK):
                              _mm(nc, ps[:, :SLW], wg[k][:, m * P:(m + 1) * P],
                                  slab[:, k, :], start=(k == 0),
                                  stop=(k == DK - 1))
                          nc.scalar.activation(gstg[:, m, :], ps[:, :SLW],
                                               ACTF.Sigmoid)
                      nc.scalar.dma_start(
                          out=gt_dram[:, :, sl * SLW:(sl + 1) * SLW], in_=gstg)
                      slab = nslab

        # Wo bf16 resident: pool created after the K/QG pools close
        wo_pool = ep(tc.tile_pool(name="wo", bufs=1))
        wo_sb = wo_pool.tile([P, DK, D], BF16)     # 16 KiB/part
        if 1 not in phases:
            nc.vector.memset(wo_sb, 0.0)

        if 1 in phases:
          # -------- V section: bf16, DMA-xbar transposed input ----------
          with tc.tile_pool(name="wvbf", bufs=1) as wvbf_pool, \
               tc.tile_pool(name="xvT", bufs=1) as xvT_pool, \
               tc.tile_pool(name="xvstage", bufs=4) as xv_pool, \
               tc.tile_pool(name="xvbf", bufs=4) as xvbf_pool, \
               tc.tile_pool(name="evict2", bufs=4, space="PSUM") as ev2_pool:
              xvT = xvT_pool.tile([P, DK, S], BF16)

              def xv_tile(s):
                  xt = xv_pool.tile([P, D], F32, tag="xvstage")
                  nc.sync.dma_start(out=xt, in_=xv[s * P:(s + 1) * P, :])
                  xb = xvbf_pool.tile([P, D], BF16, tag="xvbf")
                  nc.vector.tensor_copy(xb, xt)
                  # ~1.3us sequencer occupancy per xbar transpose: alternate
                  # queues so neither sequencer becomes the V bottleneck
                  eng = nc.scalar if s % 2 == 0 else nc.sync
                  eng.dma_start(
                      out=xvT[:, :, s * P:(s + 1) * P], in_=xb, transpose=True)

              xv_tile(0)
              xv_tile(1)
              # cast the staged Wv bits (in v_sb) to bf16
              wv_bf = wvbf_pool.tile([P, DK, D], BF16)
              for k in range(DK):
                  nc.vector.tensor_copy(
                      wv_bf[:, k, :].rearrange("p (a b) -> p a b", a=2),
                      v_sb[:, 2 * k:2 * k + 2, :].bitcast(F32))
              for s in range(2, ST):
                  xv_tile(s)

              for s in range(ST):
                  pss = [ev2_pool.tile([P, 512], F32, tag="vproj",
                                       name=f"pss{_n}") for _n in range(2)]
                  for k in range(DK):
                      for n in range(2):
                          _mm(nc, pss[n], xvT[:, k, s * P:(s + 1) * P],
                              wv_bf[:, k, n * 512:(n + 1) * 512],
                              start=(k == 0), stop=(k == DK - 1))
                  for n in range(2):
                      nc.vector.tensor_copy(
                          v_sb[:, s, n * 512:(n + 1) * 512], pss[n])

        # =================== phase 2 ===================
        if 2 not in phases:
            return

        QB = 4                      # q tiles per block
        NBLK = ST // QB
        attnp = ep(tc.tile_pool(name="attnp", bufs=2))
        outp = ep(tc.tile_pool(name="outp", bufs=3))
        blkp = ep(tc.tile_pool(name="blkp", bufs=1))
        gtp = ep(tc.tile_pool(name="gtp", bufs=2))
        qtp = ep(tc.tile_pool(name="qtp", bufs=2))
        stats = ep(tc.tile_pool(name="stats", bufs=2 * QB + 2))
        ps_a = ep(tc.tile_pool(name="ps_a", bufs=5, space="PSUM"))
        ps_b = ep(tc.tile_pool(name="ps_b", bufs=3, space="PSUM"))

        qt_tiles = {}

        def load_qt(t):
            qt_sb = qtp.tile([P, DK, P], F32R, tag="qt", name="qt_sb")
            nc.scalar.dma_start(
                out=qt_sb, in_=qt_dram[:, :, t * P:(t + 1) * P].bitcast(F32R))
            qt_tiles[t] = qt_sb

        def head(t, attnT_blk, tq):
            """scores + per-bank softmax + DMA-transpose into attnT col tq"""
            # prefetch the NEXT tile's Q slice first: the dma issue must
            # precede this tile's exp instructions on the ACT queue, which
            # wait on scores and would head-of-line block the load
            if t + 1 < ST:
                load_qt(t + 1)
            qt_sb = qt_tiles.pop(t)

            negmax4 = stats.tile([P, NB], F32, tag="negmax4", name="negmax4")
            sums4 = stats.tile([P, NB], F32, tag="sums4", name="sums4")
            neg_max = stats.tile([P, 1], F32, tag="negmax", name="neg_max")
            c4 = stats.tile([P, NB], F32, tag="c4", name="c4")
            recip = stats.tile([P, 1], F32, tag="recip", name="recip")
            sumx = stats.tile([P, 1], F32, tag="sumx", name="sumx")

            score_ps = [ps_a.tile([P, 512], F32, tag="ps_a", name=f"sps{_n}")
                        for _n in range(NB)]
            attn = attnp.tile([P, S], BF16, tag="attn", name="attn")
            # nb-major: each bank completes after 8 MMs and its softmax evict
            # starts while later banks still stream, so psum banks recycle
            # pipelined instead of all-at-once at tile end
            for nb in range(NB):
                for k in range(DK):
                    _mm(nc, score_ps[nb], qt_sb[:, k, :],
                        kT_sb[:, k, nb * 512:(nb + 1) * 512],
                        start=(k == 0), stop=(k == DK - 1))
                nc.vector.tensor_reduce(
                    negmax4[:, nb:nb + 1], score_ps[nb], axis=AX.X,
                    op=ALU.max, negate=True)
                # exp with per-bank max: frees the psum bank without waiting
                # for the global row max
                nc.scalar.activation(
                    attn[:, nb * 512:(nb + 1) * 512], score_ps[nb], ACTF.Exp,
                    bias=negmax4[:, nb:nb + 1], accum_out=sums4[:, nb:nb + 1])
            # global max + per-bank correction c4 = exp(m_nb - M)
            nc.vector.tensor_reduce(neg_max, negmax4, axis=AX.X, op=ALU.min)
            nc.vector.tensor_scalar(
                out=c4, in0=negmax4, scalar1=neg_max, scalar2=None,
                op0=ALU.subtract)
            nc.scalar.activation(c4, c4, ACTF.Exp, scale=-1.0)
            nc.vector.tensor_tensor(out=sums4, in0=sums4, in1=c4, op=ALU.mult)
            nc.vector.tensor_reduce(sumx, sums4, axis=AX.X, op=ALU.add)
            nc.vector.reciprocal(recip, sumx)
            for nb in range(NB):
                nc.vector.tensor_scalar_mul(
                    attn[:, nb * 512:(nb + 1) * 512],
                    attn[:, nb * 512:(nb + 1) * 512], c4[:, nb:nb + 1])

            # one DMA xbar transpose: attnT_blk[p, kb, tq*P+j] = attn[j, kb*P+p]
            nc.sync.dma_start(
                out=attnT_blk[:, :, tq * P:(tq + 1) * P], in_=attn,
                transpose=True)
            return recip

        def tail_block(blk, attnT_blk, recips):
            q0 = blk * QB * P           # block q offset
            gt_sb = gtp.tile([P, DK, QB * P], BF16, tag="gt", name="gt_sb")
            nc.scalar.dma_start(out=gt_sb, in_=gt_dram[:, :, q0:q0 + QB * P])

            # ctx^T = V^T x attnT (bf16), evict fused with gate^T multiply
            ctxgT_blk = blkp.tile([P, DK, QB * P], BF16, name="ctxgT_blk")
            for mp in range(DK // 2):
                ps_c = [ps_b.tile([P, 512], F32, tag="ps_b", name=f"psc{_n}")
                        for _n in range(2)]
                for kb in range(ST):
                    for h in range(2):
                        m = mp * 2 + h
                        _mm(nc, ps_c[h], v_sb[:, kb, m * P:(m + 1) * P],
                            attnT_blk[:, kb, :],
                            start=(kb == 0), stop=(kb == ST - 1))
                for h in range(2):
                    m = mp * 2 + h
                    nc.vector.tensor_tensor(
                        out=ctxgT_blk[:, m, :], in0=ps_c[h],
                        in1=gt_sb[:, m, :], op=ALU.mult)

            # out = (ctxgT x Wo) * recip, per q tile
            for tq in range(QB):
                t = blk * QB + tq
                ps_o = [ps_b.tile([P, 512], F32, tag="ps_b", name=f"pso{_n}")
                        for _n in range(2)]
                for k in range(DK):
                    for n in range(2):
                        _mm(nc, ps_o[n],
                            ctxgT_blk[:, k, tq * P:(tq + 1) * P],
                            wo_sb[:, k, n * 512:(n + 1) * 512],
                            start=(k == 0), stop=(k == DK - 1))
                for n in range(2):
                    out_sb = outp.tile([P, 512], F32, tag="out", name="out_sb")
                    nc.vector.tensor_scalar_mul(out_sb, ps_o[n], recips[tq])
                    nc.sync.dma_start(
                        out=out[t * P:(t + 1) * P, n * 512:(n + 1) * 512],
                        in_=out_sb)

        prev = None
        load_qt(0)
        # Wo load + bf16 cast, emitted in phase 2: Tile schedules by program
        # position, so this cannot steal HBM bandwidth from the V section
        wo_stage = ep(tc.tile_pool(name="wostage", bufs=1))
        for h in range(4):
            wt = wo_stage.tile([P, 2, D], F32, tag="wostage")
            nc.scalar.dma_start(
                out=wt, in_=ws["wo"][h * 2 * P:(h + 1) * 2 * P, :]
                .rearrange("(c p) d -> p c d", p=P))
            for c in range(2):
                nc.vector.tensor_copy(wo_sb[:, h * 2 + c, :], wt[:, c, :])
        for blk in range(NBLK):
            attnT_blk = blkp.tile([P, ST, QB * P], BF16, name="attnT_blk",
                                  tag=f"attnT{blk % 2}")
            recips = []
            for tq in range(QB):
                recips.append(head(blk * QB + tq, attnT_blk, tq))
            if prev is not None:
                tail_block(*prev)
            prev = (blk, attnT_blk, recips)
        tail_block(*prev)


_CACHE = {}


def _get_program(zero_bias: bool):
    if zero_bias not in _CACHE:
        _CACHE[zero_bias] = build_program(zero_bias)
    return _CACHE[zero_bias]


def kernel(queries, keys, values, Wq, bq, Wk, bk, Wv, bv, Wg, bg, Wo, bo):
    queries = np.ascontiguousarray(np.asarray(queries, dtype=np.float32))
    keys = np.ascontiguousarray(np.asarray(keys, dtype=np.float32))
    values = np.ascontiguousarray(np.asarray(values, dtype=np.float32))
    wdict = {
        "wq": np.ascontiguousarray(np.asarray(Wq, np.float32)),
        "wk": np.ascontiguousarray(np.asarray(Wk, np.float32)),
        "wv": np.ascontiguousarray(np.asarray(Wv, np.float32)),
        "wg": np.ascontiguousarray(np.asarray(Wg, np.float32)),
        "wo": np.ascontiguousarray(np.asarray(Wo, np.float32)),
    }
    bdict = {
        "bq": np.ascontiguousarray(np.asarray(bq, np.float32)),
        "bk": np.ascontiguousarray(np.asarray(bk, np.float32)),
        "bv": np.ascontiguousarray(np.asarray(bv, np.float32)),
        "bg": np.ascontiguousarray(np.asarray(bg, np.float32)),
        "bo": np.ascontiguousarray(np.asarray(bo, np.float32)),
    }
    zero_bias = all(not np.any(v) for v in bdict.values())
    if not zero_bias:
        # Bias-enabled device path is not wired up; the problem's
        # setup_inputs() uses all-zero biases, so this branch only exists
        # for off-spec inputs. Compute on host for correctness.
        return _host_reference(queries, keys, values, wdict, bdict)
    nc = _get_program(True)

    in_maps = []
    for b in range(B):
        m = {"xq": queries[b], "xk": keys[b], "xv": values[b]}
        m.update(wdict)
        in_maps.append(m)
    res = run_bass_kernel_spmd(nc, in_maps, core_ids=list(range(B)))
    return np.stack([res.results[b]["out"] for b in range(B)], axis=0)


def _host_reference(queries, keys, values, w, bdict):
    out = np.empty_like(queries)
    for b in range(B):
        q = queries[b] @ w["wq"] + bdict["bq"]
        k = keys[b] @ w["wk"] + bdict["bk"]
        v = values[b] @ w["wv"] + bdict["bv"]
        s = q @ k.T
        s -= s.max(axis=-1, keepdims=True)
        e = np.exp(s)
        a = e / e.sum(axis=-1, keepdims=True)
        gate = 1.0 / (1.0 + np.exp(-(queries[b] @ w["wg"] + bdict["bg"])))
        out[b] = ((a @ v) * gate) @ w["wo"] + bdict["bo"]
    return out
